# revision 9
# baseline (speedup 1.0000x reference)
"""Single-head attention kernel for Trainium2 (Bass/Tile), 8-core data-parallel.

Problem: h [8, 4096, 96] f32; Wq/Wk/Wv [96, 96]; bq/bk/bv [96].
  Q = h @ Wq.T + bq ; K = h @ Wk.T + bk ; V = h @ Wv.T + bv
  out = softmax(Q K^T / sqrt(96)) @ V

Sharding: batch dim across the 8 NeuronCores (1 batch element per core),
params replicated. Each core runs a flash-style attention over its
[4096, 96] slice; full output gathered on host.

Per-core design (S=4096, D=96). The kernel is ACT(exp)-bound: 16.7M exps
at 1 elem/lane/cycle @1.2GHz = 109us floor, so everything else must hide
under the exp stream; ACT's ~222-cycle per-instruction access bubble is
amortized with wide ACTIVATEs over multi-bank PSUM reads.

  - A-trick: S*sqrt(D) = Q K^T = h~ (W~q W~k^T) h~^T with h~ = [h, 1].
    A~ = W~q W~k^T / sqrt(D) is a single tiny on-chip matmul of the
    augmented weights ([Wq | bq-col] etc.); G~^T = A~^T h~^T replaces
    separate Q and K projections (one 4096-row projection, not two).
  - All big matmuls run fp16 operands (1 cycle/row at any free size; f32
    PSUM dst as TRN2 requires). Rel err ~4e-4.
  - scores^T tile [j, i] = (h~^T j-slab).T @ G~^T i-cols, in i-chunks of
    512. Alternating groups of 2/3 j-tiles share a 2-bank/3-bank PSUM
    slot pair so each exp ACTIVATE covers N=1024/1536.
  - exp -> e_sb ring (fp16, 2-chunk parity) feeds PV as the *moving*
    operand: acc^T [97, 512] += V~_j.T @ e_slice [128, 512], V~ tiles
    stationary. 512-row moving MMs fully hide the ~100ns LDWEIGHTS (a
    [128,128]-stationary PV variant measured 104ns/LDW exposed). V~'s
    ones column (from the projection) makes acc row 96 the denominator.
  - Epilogue per chunk: acc -> SBUF copy, 4 PE transposes [97,128] ->
    [128,97], DVE reciprocal + per-row mul, DMA out. PSUM budget: 2+3
    (scores) + 2 (acc ping-pong) + 1 (transpose/G-seg) = 8 banks.
  - Software pipeline: PV of chunk c runs *inside* chunk c lagging the
    exp stream by 2 groups (the 2-deep acc ping-pong makes the chunk
    handover seamless and the tail short); its epilogue lands in chunk
    c+1 groups 2-6. h-slab transposes (batched 4 per PSUM round-trip)
    and the V~ projection ride chunk 0; G~^T segment c+1 rides chunk c.
    h-tile DMAs are issued ahead of everything else (dma_start costs
    ~600ns on the serial Sync queue - ordering it badly adds ~6us of
    dead prologue).
"""

import functools
import math

import numpy as np

import concourse.mybir as mybir
import concourse.tile as tile
from concourse import bacc
from concourse.bass import ts
from concourse.bass_utils import run_bass_kernel_spmd

S = 4096
D = 96
P = 128              # j/i tile (partition) size
NI = 512             # i-chunk width (columns per scores matmul)
N_CORES = 8
F32 = mybir.dt.float32
F16 = mybir.dt.float16
AF = mybir.ActivationFunctionType

# Exp-group widths (j-tiles) per chunk: alternating 2/3 = 13 groups, 32
# tiles. Even groups -> 2-bank slot, odd -> 3-bank slot.
GW = [2, 3] * 6 + [2]
GSTART = [sum(GW[:g]) for g in range(len(GW))]
NG = len(GW)


def jgroup(j):
    p, r = divmod(j, 5)
    return (2 * p, r) if r < 2 else (2 * p + 1, r - 2)


def build_attention_kernel(tc, out_dram, h, Wq, bq, Wk, bk, Wv, bv, s=S):
    nc = tc.nc
    nj = s // P                    # 32 j-tiles
    nchunks = s // NI              # 8 i-chunks
    ntile = NI // P                # 4 i-tiles per chunk
    scale = 1.0 / math.sqrt(D)

    from contextlib import ExitStack
    with ExitStack() as ctx:
        singles = ctx.enter_context(tc.tile_pool(name="singles", bufs=1))
        hp = ctx.enter_context(tc.tile_pool(name="hp", bufs=8))
        op = ctx.enter_context(tc.tile_pool(name="op", bufs=4))
        # PSUM: 3+2-bank score slots + 2 x 1-bank acc + 1-bank misc = 8.
        scpA = ctx.enter_context(
            tc.tile_pool(name="scpA", bufs=1, space="PSUM"))
        scpB = ctx.enter_context(
            tc.tile_pool(name="scpB", bufs=1, space="PSUM"))
        accp = ctx.enter_context(
            tc.tile_pool(name="accp", bufs=2, space="PSUM"))
        trp = ctx.enter_context(tc.tile_pool(name="trp", bufs=1, space="PSUM"))

        # h-tile DMAs for the first transpose batch go FIRST (Sync queue
        # is serial at ~600ns/DMA; h gates the whole pipeline).
        h_sb0 = []
        for k in range(4):
            h_sb = hp.tile([P, D], F32, tag="h_sb")
            nc.sync.dma_start(out=h_sb, in_=h[ts(k, P), :])
            h_sb0.append(h_sb)
        ident_dram = nc.inline_tensor(np.eye(P, dtype=np.float32),
                                      name="ident_const")
        ident = singles.tile([P, P], F32)
        nc.sync.dma_start(out=ident, in_=ident_dram.ap())

        # --- persistent SBUF tensors ---
        hT = singles.tile([D + 1, s], F16)        # h~^T (row 96 = ones)
        GT = singles.tile([D + 1, s], F16)        # G~^T = A~^T h~^T
        Vt = singles.tile([P, nj, D + 1], F16)    # V~ tiles (col 96 = ones)
        # e_sb ring: [chunk parity][group][<=3*NI cols] of exp scores^T
        e_sb = singles.tile([P, 2, NG, 3 * NI], F16)
        ones_col = singles.tile([P, 1], F32)

        # --- weights: augmented transposed forms, no host-side prep ---
        # waq/wak [96, 97] f32: cols 0-95 = Wq/Wk (natural), col 96 = bias.
        # R [97, 97] fp16: rows 0-95 = Wv^T, row 96 = bv, col 96 = e_96.
        waq = singles.tile([D, D + 1], F32)
        wak = singles.tile([D, D + 1], F32)
        wv_sb = singles.tile([D, D], F32)
        bias_sb = singles.tile([2, D], F32)
        bv_sb = singles.tile([1, D], F32)
        nc.sync.dma_start(out=waq[:, 0:D], in_=Wq)
        nc.sync.dma_start(out=wak[:, 0:D], in_=Wk)
        nc.sync.dma_start(out=bias_sb[0:1, :], in_=bq.unsqueeze(0))
        nc.sync.dma_start(out=bias_sb[1:2, :], in_=bk.unsqueeze(0))
        ones_dram = nc.inline_tensor(np.ones((1, s), dtype=np.float16),
                                     name="ones_row")
        nc.sync.dma_start(out=hT[D:D + 1, :], in_=ones_dram.ap())
        nc.sync.dma_start(out=wv_sb, in_=Wv)
        nc.sync.dma_start(out=bv_sb, in_=bv.unsqueeze(0))
        nc.vector.memset(ones_col, 1.0)

        # bias columns via one PE transpose of [2, 96]
        ps_b = trp.tile([D, 2], F32, tag="u")
        nc.tensor.transpose(ps_b, bias_sb, ident[0:2, 0:2])
        nc.vector.tensor_copy(waq[:, D:D + 1], ps_b[:, 0:1])
        nc.vector.tensor_copy(wak[:, D:D + 1], ps_b[:, 1:2])

        # A~ = (W~q W~k^T) * scale -> fp16 [97, 97]
        ps_a = accp.tile([D + 1, D + 1], F32, tag="u")
        nc.tensor.matmul(ps_a, lhsT=waq, rhs=wak, start=True, stop=True)
        A16 = singles.tile([D + 1, D + 1], F16)
        nc.vector.tensor_scalar_mul(A16, ps_a, scale)

        # R for the V~ projection
        R = singles.tile([D + 1, D + 1], F16)
        nc.vector.memset(R, 0.0)
        ps_w = trp.tile([D, D], F32, tag="u")
        nc.tensor.transpose(ps_w, wv_sb, ident[0:D, 0:D])
        nc.vector.tensor_copy(R[0:D, 0:D], ps_w)
        nc.vector.tensor_copy(R[D:D + 1, 0:D], bv_sb)
        nc.vector.tensor_copy(R[D:D + 1, D:D + 1], ones_col[0:1, 0:1])

        # --- batched prologue helpers (4 tiles per PSUM round-trip) ---
        # Pool choice is static: everything allocated from accp must
        # precede acc(0)'s allocation (chunk 0 group 3) in rotation
        # order, or the pinned in-flight accumulator deadlocks the pool.

        def emit_tr4(q, pool, pre=None):
            # transpose h j-tiles 4q..4q+3 into hT via one wide PSUM tile
            pt = pool.tile([D, 4 * P], F32, tag="u")
            for k in range(4):
                if pre is not None:
                    h_sb = pre[k]
                else:
                    h_sb = hp.tile([P, D], F32, tag="h_sb")
                    nc.sync.dma_start(out=h_sb, in_=h[ts(4 * q + k, P), :])
                nc.tensor.transpose(pt[:, ts(k, P)], h_sb, ident)
            nc.vector.tensor_copy(hT[0:D, ts(q, 4 * P)], pt)

        def emit_v4(q, pool):
            # V~ projection for j-tiles 4q..4q+3
            pt = pool.tile([P, 4 * (D + 1)], F32, tag="u")
            for k in range(4):
                nc.tensor.matmul(pt[:, ts(k, D + 1)],
                                 lhsT=hT[:, ts(4 * q + k, P)], rhs=R,
                                 start=True, stop=True)
            nc.vector.tensor_copy(Vt[:, 4 * q:4 * q + 4, :], pt)

        def emit_g_seg(seg):
            ps_g = trp.tile([D + 1, NI], F32, tag="u")
            nc.tensor.matmul(ps_g, lhsT=A16, rhs=hT[:, ts(seg, NI)],
                             start=True, stop=True)
            nc.vector.tensor_copy(GT[:, ts(seg, NI)], ps_g)

        # PV + epilogue of chunk c as (absolute-group, unit); group 0 of
        # chunk c is absolute c*NG. PV MM for j lands 4 groups after its
        # exp group (units created lazily at chunk c group 3 so acc's
        # pool turn comes after all of chunk 0's prologue tiles); the
        # epilogue spans chunk c+1 groups 4-8.
        def pv_units(c):
            par = c % 2
            units = []
            acc = accp.tile([D + 1, NI], F32, tag="u")
            for j in range(nj):
                g, jj = jgroup(j)
                units.append((c * NG + g + 4, functools.partial(
                    nc.tensor.matmul, acc,
                    lhsT=Vt[:, j, :],
                    rhs=e_sb[:, par, g, ts(jj, NI)],
                    start=(j == 0), stop=(j == nj - 1))))
            eoT = op.tile([D + 1, NI], F32, tag="eoT")
            units.append((c * NG + 17,
                          functools.partial(nc.vector.tensor_copy, eoT, acc)))

            def epi(k, c=c, eoT=eoT):
                ps_tr = trp.tile([P, D + 1], F32, tag="u")
                nc.tensor.transpose(ps_tr, eoT[:, ts(k, P)],
                                    ident[0:D + 1, 0:D + 1])
                rec = op.tile([P, 1], F32, tag="rec")
                nc.vector.reciprocal(rec, ps_tr[:, D:D + 1])
                o_sb = op.tile([P, D], F32, tag="o_sb")
                nc.vector.tensor_scalar_mul(o_sb, ps_tr[:, 0:D], rec)
                i0 = c * NI + k * P
                nc.sync.dma_start(out=out_dram[i0:i0 + P, :], in_=o_sb)
            for k in range(ntile):
                units.append((c * NG + 18 + k, functools.partial(epi, k)))
            return units

        # --- prologue: enough for chunk 0 group 0 ---
        emit_tr4(0, accp, pre=h_sb0)
        emit_g_seg(0)
        # chunk-0 extras schedule: (group -> list of emitters). accp uses
        # all precede group 3 (before acc(0)); the rest go through trp.
        extras0 = {
            0: [lambda: emit_tr4(1, accp), lambda: emit_v4(0, trp)],
            1: [lambda: emit_tr4(2, accp), lambda: emit_v4(1, trp)],
            2: [lambda: emit_tr4(3, accp), lambda: emit_v4(2, trp)],
            3: [lambda: emit_v4(3, accp)],
            4: [lambda: emit_tr4(4, trp)],
            5: [lambda: emit_tr4(5, trp), lambda: emit_v4(4, trp)],
            6: [lambda: emit_tr4(6, trp), lambda: emit_v4(5, trp)],
            7: [lambda: emit_tr4(7, trp), lambda: emit_v4(6, trp)],
            8: [lambda: emit_v4(7, trp)],
        }

        # --- main loop ---
        pending = []          # (abs_group, unit) for PV interleave
        abs_g = 0
        for c in range(nchunks):
            for g in range(NG):
                w = GW[g]
                pool = scpA if w == 3 else scpB
                sc = pool.tile([P, w * NI], F32, tag="sc")
                for jj in range(w):
                    nc.tensor.matmul(sc[:, ts(jj, NI)],
                                     lhsT=hT[:, ts(GSTART[g] + jj, P)],
                                     rhs=GT[:, ts(c, NI)],
                                     start=True, stop=True)
                nc.scalar.activation(out=e_sb[:, c % 2, g, 0:w * NI],
                                     in_=sc, func=AF.Exp)
                # interleaved extras keep PE dense without starving ACT
                if c == 0:
                    for emitter in extras0.get(g, []):
                        emitter()
                if g == 8 and c + 1 < nchunks:
                    emit_g_seg(c + 1)
                if g == 3:
                    pending.extend(pv_units(c))
                left = []
                for ag, u in pending:
                    (u() if ag <= abs_g else left.append((ag, u)))
                pending = left
                abs_g += 1
        # tail: drain the last chunk's PV + epilogue
        for _, u in sorted(pending, key=lambda x: x[0]):
            u()


@functools.lru_cache(maxsize=None)
def _build_module(s=S):
    nc = bacc.Bacc("TRN2", target_bir_lowering=False, debug=False,
                   num_devices=N_CORES)
    h = nc.dram_tensor("h", [s, D], F32, kind="ExternalInput").ap()
    Wq = nc.dram_tensor("Wq", [D, D], F32, kind="ExternalInput").ap()
    bq = nc.dram_tensor("bq", [D], F32, kind="ExternalInput").ap()
    Wk = nc.dram_tensor("Wk", [D, D], F32, kind="ExternalInput").ap()
    bk = nc.dram_tensor("bk", [D], F32, kind="ExternalInput").ap()
    Wv = nc.dram_tensor("Wv", [D, D], F32, kind="ExternalInput").ap()
    bv = nc.dram_tensor("bv", [D], F32, kind="ExternalInput").ap()
    out = nc.dram_tensor("out", [s, D], F32, kind="ExternalOutput").ap()
    with tile.TileContext(nc) as tc:
        build_attention_kernel(tc, out, h, Wq, bq, Wk, bk, Wv, bv, s=s)
    nc.compile()
    return nc


def _run(inputs, trace=False):
    nc = _build_module(S)
    arrs = {k: np.ascontiguousarray(np.asarray(v), dtype=np.float32)
            for k, v in inputs.items()}
    in_maps = []
    for b_ in range(N_CORES):
        in_maps.append({
            "h": arrs["h"][b_],
            "Wq": arrs["Wq"], "bq": arrs["bq"],
            "Wk": arrs["Wk"], "bk": arrs["bk"],
            "Wv": arrs["Wv"], "bv": arrs["bv"],
        })
    res = run_bass_kernel_spmd(nc, in_maps, core_ids=list(range(N_CORES)),
                               trace=trace)
    out = np.stack([res.results[b_]["out"] for b_ in range(N_CORES)], axis=0)
    return out, res


def kernel(**inputs):
    out, _ = _run(inputs, trace=False)
    return out


def kernel_profiled(trace=True, **inputs):
    out, res = _run(inputs, trace=trace)
    return out, res


# revision 11
# speedup vs baseline: 1.0795x; 1.0795x over previous
"""Single-head attention kernel for Trainium2 (Bass/Tile), 8-core data-parallel.

Problem: h [8, 4096, 96] f32; Wq/Wk/Wv [96, 96]; bq/bk/bv [96].
  Q = h @ Wq.T + bq ; K = h @ Wk.T + bk ; V = h @ Wv.T + bv
  out = softmax(Q K^T / sqrt(96)) @ V

Sharding: batch dim across the 8 NeuronCores (1 batch element per core),
params replicated. Each core runs a flash-style attention over its
[4096, 96] slice; full output gathered on host.

Per-core design (S=4096, D=96). The kernel is ACT(exp)-bound: 16.7M exps
at 1 elem/lane/cycle @1.2GHz = 109us floor, so everything else must hide
under the exp stream; ACT's ~222-cycle per-instruction access bubble is
amortized with wide ACTIVATEs over 3-bank PSUM reads (N=1536).

  - A-trick: S*sqrt(D) = Q K^T = h~ (W~q W~k^T) h~^T with h~ = [h, 1].
    A~ = W~q W~k^T / sqrt(D) is a single tiny on-chip matmul of the
    augmented weights ([Wq | bq-col] etc.); G~^T = A~^T h~^T replaces
    separate Q and K projections (one 4096-row projection, not two).
  - All big matmuls run fp16 operands (1 cycle/row at any free size; f32
    PSUM dst as TRN2 requires). Rel err ~4e-4.
  - scores^T tile [j, i] = (h~^T j-slab).T @ G~^T i-cols, in i-chunks of
    512. Groups of 3 j-tiles share one 3-bank PSUM slot; 2 slots
    ping-pong; one exp ACTIVATE per group.
  - exp -> e_sb ring (fp16, 2-chunk parity) feeds PV as the *moving*
    operand: acc^T [97, 512] += V~_j.T @ e_slice [128, 512], V~ tiles
    stationary. 512-row moving MMs fully hide the ~100ns LDWEIGHTS (a
    [128,128]-stationary PV variant measured 104ns/LDW exposed). V~'s
    ones column (from the projection) makes acc row 96 the denominator.
  - Epilogue per chunk: acc -> SBUF copy, 4 PE transposes [97,128] ->
    [128,97], DVE reciprocal + per-row mul, DMA out. PSUM budget: 2x3
    (scores) + 1 (acc) + 1 (transpose/G-seg) = 8 banks exactly.
  - Software pipeline: PV of chunk c-1 + its epilogue interleave into
    chunk c's score stream (MMs 5/group over groups 0-6, copy at 7,
    transposes at 8/9/10/next-0); h-slab transposes (batched 4 per PSUM
    round-trip AND 4 per DMA instruction - dma_start costs ~600ns on
    the serial Sync queue) and the V~ projection ride chunk 0; G~^T
    segment c+1 rides chunk c. Prologue DMA order is chosen so the
    A~ -> G~^T -> scores chain starts as early as possible.
"""

import functools
import math

import numpy as np

import concourse.mybir as mybir
import concourse.tile as tile
from concourse import bacc
from concourse.bass import ts
from concourse.bass_utils import run_bass_kernel_spmd

S = 4096
D = 96
P = 128              # j/i tile (partition) size
NI = 512             # i-chunk width (columns per scores matmul)
JG = 3               # j-tiles per exp group (3 PSUM banks -> N=1536)
N_CORES = 8
F32 = mybir.dt.float32
F16 = mybir.dt.float16
AF = mybir.ActivationFunctionType


def build_attention_kernel(tc, out_dram, h, Wq, bq, Wk, bk, Wv, bv, s=S):
    nc = tc.nc
    nj = s // P                    # 32 j-tiles
    nchunks = s // NI              # 8 i-chunks
    ntile = NI // P                # 4 i-tiles per chunk
    njg = (nj + JG - 1) // JG      # 11 exp groups per chunk (last ragged)
    scale = 1.0 / math.sqrt(D)

    def jtiles(g):
        return range(g * JG, min((g + 1) * JG, nj))

    from contextlib import ExitStack
    with ExitStack() as ctx:
        singles = ctx.enter_context(tc.tile_pool(name="singles", bufs=1))
        hp = ctx.enter_context(tc.tile_pool(name="hp", bufs=3))
        op = ctx.enter_context(tc.tile_pool(name="op", bufs=4))
        # PSUM: 2 x 3-bank score slots + 1-bank acc + 1-bank misc = 8.
        scp = ctx.enter_context(tc.tile_pool(name="scp", bufs=2, space="PSUM"))
        accp = ctx.enter_context(
            tc.tile_pool(name="accp", bufs=1, space="PSUM"))
        trp = ctx.enter_context(tc.tile_pool(name="trp", bufs=1, space="PSUM"))

        # --- persistent SBUF tensors ---
        hT = singles.tile([D + 1, s], F16)        # h~^T (row 96 = ones)
        GT = singles.tile([D + 1, s], F16)        # G~^T = A~^T h~^T
        Vt = singles.tile([P, nj, D + 1], F16)    # V~ tiles (col 96 = ones)
        # e_sb ring: [chunk parity][group][JG*NI cols] of exp(scores^T) fp16
        e_sb = singles.tile([P, 2, njg, JG * NI], F16)
        ones_col = singles.tile([P, 1], F32)
        waq = singles.tile([D, D + 1], F32)
        wak = singles.tile([D, D + 1], F32)
        wv_sb = singles.tile([D, D], F32)
        bias_sb = singles.tile([2, D], F32)
        bv_sb = singles.tile([1, D], F32)

        # --- prologue DMAs, ordered for the critical chain:
        # ident -> h batch 0 (transposes) -> Wq/Wk/biases (A~ chain) ->
        # ones row (G needs hT row 96) -> the rest.
        ident_dram = nc.inline_tensor(np.eye(P, dtype=np.float32),
                                      name="ident_const")
        ident = singles.tile([P, P], F32)
        nc.sync.dma_start(out=ident, in_=ident_dram.ap())

        def dma_h4(q):
            # one DMA instruction for h j-tiles 4q..4q+3 -> [128, 4, 96]
            h4 = hp.tile([P, 4, D], F32, tag="h4")
            nc.sync.dma_start(
                out=h4,
                in_=h[ts(q, 4 * P), :].rearrange("(k p) e -> p k e", k=4))
            return h4

        h4_0 = dma_h4(0)
        nc.sync.dma_start(out=waq[:, 0:D], in_=Wq)
        nc.sync.dma_start(out=wak[:, 0:D], in_=Wk)
        nc.sync.dma_start(out=bias_sb[0:1, :], in_=bq.unsqueeze(0))
        nc.sync.dma_start(out=bias_sb[1:2, :], in_=bk.unsqueeze(0))
        ones_dram = nc.inline_tensor(np.ones((1, s), dtype=np.float16),
                                     name="ones_row")
        nc.sync.dma_start(out=hT[D:D + 1, :], in_=ones_dram.ap())
        nc.sync.dma_start(out=wv_sb, in_=Wv)
        nc.sync.dma_start(out=bv_sb, in_=bv.unsqueeze(0))
        nc.vector.memset(ones_col, 1.0)

        # bias columns via one PE transpose of [2, 96]
        ps_b = trp.tile([D, 2], F32, tag="u")
        nc.tensor.transpose(ps_b, bias_sb, ident[0:2, 0:2])
        nc.vector.tensor_copy(waq[:, D:D + 1], ps_b[:, 0:1])
        nc.vector.tensor_copy(wak[:, D:D + 1], ps_b[:, 1:2])

        # A~ = (W~q W~k^T) * scale -> fp16 [97, 97]
        ps_a = accp.tile([D + 1, D + 1], F32, tag="u")
        nc.tensor.matmul(ps_a, lhsT=waq, rhs=wak, start=True, stop=True)
        A16 = singles.tile([D + 1, D + 1], F16)
        nc.vector.tensor_scalar_mul(A16, ps_a, scale)

        # R for the V~ projection
        R = singles.tile([D + 1, D + 1], F16)
        nc.vector.memset(R, 0.0)
        ps_w = trp.tile([D, D], F32, tag="u")
        nc.tensor.transpose(ps_w, wv_sb, ident[0:D, 0:D])
        nc.vector.tensor_copy(R[0:D, 0:D], ps_w)
        nc.vector.tensor_copy(R[D:D + 1, 0:D], bv_sb)
        nc.vector.tensor_copy(R[D:D + 1, D:D + 1], ones_col[0:1, 0:1])

        # --- batched prologue helpers (4 tiles per PSUM round-trip) ---
        pools = [accp, trp]

        def emit_tr4(q, pre=None):
            # transpose h j-tiles 4q..4q+3 into hT via one wide PSUM tile
            h4 = pre if pre is not None else dma_h4(q)
            pt = pools[q % 2].tile([D, 4 * P], F32, tag="u")
            for k in range(4):
                nc.tensor.transpose(pt[:, ts(k, P)], h4[:, k, :], ident)
            nc.vector.tensor_copy(hT[0:D, ts(q, 4 * P)], pt)

        def emit_v4(q):
            # V~ projection for j-tiles 4q..4q+3
            pt = pools[(q + 1) % 2].tile([P, 4 * (D + 1)], F32, tag="u")
            for k in range(4):
                nc.tensor.matmul(pt[:, ts(k, D + 1)],
                                 lhsT=hT[:, ts(4 * q + k, P)], rhs=R,
                                 start=True, stop=True)
            nc.vector.tensor_copy(Vt[:, 4 * q:4 * q + 4, :], pt)

        def emit_g_seg(seg):
            ps_g = trp.tile([D + 1, NI], F32, tag="u")
            nc.tensor.matmul(ps_g, lhsT=A16, rhs=hT[:, ts(seg, NI)],
                             start=True, stop=True)
            nc.vector.tensor_copy(GT[:, ts(seg, NI)], ps_g)

        # PV + epilogue of chunk c as (group-offset-in-next-chunk, unit)
        def pv_units(c):
            par = c % 2
            units = []
            acc = accp.tile([D + 1, NI], F32, tag="u")
            for j in range(nj):
                g, jj = j // JG, j % JG
                units.append((j // 5, functools.partial(
                    nc.tensor.matmul, acc,
                    lhsT=Vt[:, j, :],
                    rhs=e_sb[:, par, g, ts(jj, NI)],
                    start=(j == 0), stop=(j == nj - 1))))
            eoT = op.tile([D + 1, NI], F32, tag="eoT")
            units.append((7, functools.partial(nc.vector.tensor_copy,
                                               eoT, acc)))

            def epi(k, c=c, eoT=eoT):
                ps_tr = trp.tile([P, D + 1], F32, tag="u")
                nc.tensor.transpose(ps_tr, eoT[:, ts(k, P)],
                                    ident[0:D + 1, 0:D + 1])
                rec = op.tile([P, 1], F32, tag="rec")
                nc.vector.reciprocal(rec, ps_tr[:, D:D + 1])
                o_sb = op.tile([P, D], F32, tag="o_sb")
                nc.vector.tensor_scalar_mul(o_sb, ps_tr[:, 0:D], rec)
                i0 = c * NI + k * P
                nc.sync.dma_start(out=out_dram[i0:i0 + P, :], in_=o_sb)
            for k in range(ntile):
                units.append((8 + k, functools.partial(epi, k)))
            return units

        # --- prologue: enough for chunk 0 group 0 ---
        emit_tr4(0, pre=h4_0)
        emit_g_seg(0)

        # --- main loop ---
        pending = []          # (abs_group, unit) for PV interleave
        abs_g = 0
        for c in range(nchunks):
            if c > 0:
                pending.extend((abs_g + off, u) for off, u in pv_units(c - 1))
            for g in range(njg):
                jts = list(jtiles(g))
                sc = scp.tile([P, JG * NI], F32, tag="sc")
                for jj, jt in enumerate(jts):
                    nc.tensor.matmul(sc[:, ts(jj, NI)],
                                     lhsT=hT[:, ts(jt, P)],
                                     rhs=GT[:, ts(c, NI)],
                                     start=True, stop=True)
                width = len(jts) * NI
                nc.scalar.activation(out=e_sb[:, c % 2, g, 0:width],
                                     in_=sc[:, 0:width], func=AF.Exp)
                # interleaved extras keep PE dense without starving ACT
                if c == 0:
                    if g < 7:
                        emit_tr4(g + 1)
                    if g < 8:
                        emit_v4(g)
                if g == 5 and c + 1 < nchunks:
                    emit_g_seg(c + 1)
                left = []
                for ag, u in pending:
                    (u() if ag <= abs_g else left.append((ag, u)))
                pending = left
                abs_g += 1
        # tail: drain pending epilogue, then PV of the last chunk
        for _, u in sorted(pending, key=lambda x: x[0]):
            u()
        for _, u in pv_units(nchunks - 1):
            u()


@functools.lru_cache(maxsize=None)
def _build_module(s=S):
    nc = bacc.Bacc("TRN2", target_bir_lowering=False, debug=False,
                   num_devices=N_CORES)
    h = nc.dram_tensor("h", [s, D], F32, kind="ExternalInput").ap()
    Wq = nc.dram_tensor("Wq", [D, D], F32, kind="ExternalInput").ap()
    bq = nc.dram_tensor("bq", [D], F32, kind="ExternalInput").ap()
    Wk = nc.dram_tensor("Wk", [D, D], F32, kind="ExternalInput").ap()
    bk = nc.dram_tensor("bk", [D], F32, kind="ExternalInput").ap()
    Wv = nc.dram_tensor("Wv", [D, D], F32, kind="ExternalInput").ap()
    bv = nc.dram_tensor("bv", [D], F32, kind="ExternalInput").ap()
    out = nc.dram_tensor("out", [s, D], F32, kind="ExternalOutput").ap()
    with tile.TileContext(nc) as tc:
        build_attention_kernel(tc, out, h, Wq, bq, Wk, bk, Wv, bv, s=s)
    nc.compile()
    return nc


def _run(inputs, trace=False):
    nc = _build_module(S)
    arrs = {k: np.ascontiguousarray(np.asarray(v), dtype=np.float32)
            for k, v in inputs.items()}
    in_maps = []
    for b_ in range(N_CORES):
        in_maps.append({
            "h": arrs["h"][b_],
            "Wq": arrs["Wq"], "bq": arrs["bq"],
            "Wk": arrs["Wk"], "bk": arrs["bk"],
            "Wv": arrs["Wv"], "bv": arrs["bv"],
        })
    res = run_bass_kernel_spmd(nc, in_maps, core_ids=list(range(N_CORES)),
                               trace=trace)
    out = np.stack([res.results[b_]["out"] for b_ in range(N_CORES)], axis=0)
    return out, res


def kernel(**inputs):
    out, _ = _run(inputs, trace=False)
    return out


def kernel_profiled(trace=True, **inputs):
    out, res = _run(inputs, trace=trace)
    return out, res
